# revision 1
# baseline (speedup 1.0000x reference)
"""Trainium2 Bass kernel for nn_AttentionLayer (B=4, C=256, N=4096, CR=32).

Sharding: 8 cores = (batch b in 0..3) x (query-half ih in 0..1).
Each core receives x[b] rotated so its own query half sits at columns
0..2047 (softmax is invariant to key order, so the rotation is exact);
it computes out[b][:, ih*2048:(ih+1)*2048] and the host reassembles.

Per-core algorithm (dtype float32r = TF32-class PE inputs, fp32 PSUM):
  - stacked 1x1 conv [Wk; bv@Wk; pad; Wq] @ x -> g (keys), gbv row
    (query-bias correction), h (values)
  - f = Wv @ xq + bv (queries, own half = x columns 0..2047)
  - scores s^T[j,i] = sum_c g_aug[c,j] * f_aug[c,i], K=33 augmented
    contraction ([f;1] x [g;gbv]) so s already includes the query bias.
    Key bias bk is constant over j -> cancels in softmax -> dropped.
    Value bias bq is folded into the output conv bias on the host.
  - exp on ACT, no max subtraction (|s| <~ 40 fits fp32 range)
  - num/den via one accumulating matmul with lhsT = [h^T | 1] per j-tile
  - reciprocal(den), broadcast over partitions via a PE ones-matmul
  - out = (gamma*Wo) @ (num*rden) + (gamma*(Wo@bq + bo)) + x  (bias via
    an exact-ones row in the rhs and a bias row in the weights; the
    residual reads the f32r x tile bitcast back to f32, so it is exact)
"""

import numpy as np

B, C, N = 4, 256, 4096
CR = 32
NH = N // 2          # queries per core
G = 512              # i-group width
NCORES = 8

_CACHE = {}


def build_program():
    """Build the (shared, SPMD) Bass program. Returns compiled nc."""
    import concourse.bacc as bacc
    import concourse.mybir as mybir
    from concourse.tile import TileContext

    dt = mybir.dt
    f32 = dt.float32
    f32r = dt.float32r
    Exp = mybir.ActivationFunctionType.Exp
    add = mybir.AluOpType.add
    mult = mybir.AluOpType.mult

    nc = bacc.Bacc("TRN2", target_bir_lowering=False, debug=False,
                   num_devices=NCORES)

    # --- I/O (all PE operands declared f32r; host passes fp32 bits) ---
    # xw: host-packed, laid out identically to the SBUF tile so every DMA
    # is a dense contiguous copy: [wght(192) | x piece 0 (1024) |
    # wft(64), wot(256, rows 0-63), e0(32), idm(32, rows 0-31) |
    # x pieces 1-7 (7x1024)].  x piece gp holds chunks c=0,1 of columns
    # [gp*512, (gp+1)*512) side by side.  DMA 0 carries exactly what the
    # first conv needs (wght + piece 0).
    xw = nc.dram_tensor("xw", [128, 1600 + 7 * 1024], f32r,
                        kind="ExternalInput").ap()
    res = nc.dram_tensor("res", [C, NH], f32, kind="ExternalOutput").ap()

    NJT = N // 128            # 32 j-tiles
    NIG = NH // G             # 4 i-groups
    SUPERS = [3, 3, 3, 3, 3, 3, 3, 3, 3, 3, 2]   # j-tiles per super (sum 32)
    assert sum(SUPERS) == NJT

    with TileContext(nc) as tc:
        with (
            tc.tile_pool(name="const", bufs=1) as cpool,
            tc.tile_pool(name="big", bufs=1) as bpool,
            tc.tile_pool(name="eb", bufs=6) as epool,
            tc.tile_pool(name="small", bufs=2) as spool,
            tc.tile_pool(name="resp", bufs=3) as rpool,
            tc.tile_pool(name="psA", bufs=1, space="PSUM") as psA,
            tc.tile_pool(name="psB", bufs=1, space="PSUM") as psB,
            tc.tile_pool(name="pso", bufs=1, space="PSUM") as pso,
            tc.tile_pool(name="pst", bufs=1, space="PSUM") as pst,
        ):
            # --- weights + x in one identity-layout tile; DMA 0 carries
            # the weights together with x piece 0 ---
            # layout: [wght(192) | piece0(1024) | wft,wot,e0,idm(384) |
            # pieces 1-7].  DMA0 carries exactly what conv0 needs.
            xall = bpool.tile([128, 1600 + 7 * 1024], f32r)
            wght_t = xall[:, 0:192]
            wft_t = xall[:, 1216:1280]
            wot_t = xall[0:64, 1280:1536]
            e0_t = xall[:, 1536:1568]
            idm_t = xall[0:32, 1568:1600]
            nc.sync.dma_start(xall[:, 0:1216], xw[:, 0:1216])
            nc.sync.dma_start(xall[:, 1216:1600], xw[:, 1216:1600])
            for gp in range(1, 8):
                s0 = 1600 + (gp - 1) * 1024
                nc.sync.dma_start(xall[:, s0:s0 + 1024], xw[:, s0:s0 + 1024])

            def xv(c, col, w):
                # x chunk c, columns [col, col+w) in piece-major layout
                gp = col // G
                assert col % G + w <= G
                base = 192 if gp == 0 else 1600 + (gp - 1) * 1024
                return xall[:, base + c * G + col % G:
                            base + c * G + col % G + w]

            # --- activation buffers ---
            f_t = []
            for gi in range(NH // G):
                ft = bpool.tile([128, G], f32r, name=f"f{gi}")
                f_t.append(ft)
                nc.vector.memset(ft[32:33, :].bitcast(f32), 1.0)
            g_aug = bpool.tile([128, N], f32r)    # rows: g(32), gbv(1)
            h_sb = bpool.tile([128, N], f32r)     # rows: h(32)
            hpt = bpool.tile([128, NJT * 33], f32r)  # [h^T | 1] per j-tile

            SPOOLS = (psA, psB)
            SNAMES = ("sa", "sb")

            # --- stacked gh conv: psum rows [g(32); gbv(1); pad; h@64] ---
            GC = 512

            def emit_gh_conv(grp):
                cps = pst.tile([128, GC], f32, name="tl")
                for c in range(2):
                    nc.tensor.matmul(
                        cps[0:96, :],
                        wght_t[:, c * 96:(c + 1) * 96],
                        xv(c, grp * GC, GC),
                        start=(c == 0), stop=(c == 1))
                sl = slice(grp * GC, (grp + 1) * GC)
                nc.vector.tensor_copy(g_aug[0:33, sl], cps[0:33, :])
                nc.vector.tensor_copy(h_sb[0:32, sl], cps[64:96, :])
                emit_gh_tps(grp)

            # transpose a group's 2 h j-tiles into hpt
            def emit_gh_tps(grp):
                tps = pst.tile([128, 128], f32r, name="tlt", tag="tl")
                for k in range(4):
                    t = 4 * grp + k
                    nc.tensor.transpose(
                        tps[:, k * 32:(k + 1) * 32],
                        h_sb[0:32, t * 128:(t + 1) * 128],
                        idm_t)
                hpt_v = hpt[:].rearrange("p (t w) -> p t w", w=33)
                nc.vector.tensor_copy(
                    hpt_v[:, 4 * grp:4 * grp + 4, 0:32],
                    tps[:].rearrange("p (t w) -> p t w", w=32))
                nc.vector.memset(hpt_v[:, 4 * grp:4 * grp + 4, 32:33].bitcast(f32), 1.0)

            # --- f conv (own query half): f = Wv @ xq (bias via gbv row) ---
            def emit_f_conv(fg, pool=None, name="tl"):
                cps = (pool or pst).tile([128, G], f32, name=name, tag=name)
                for c in range(2):
                    nc.tensor.matmul(
                        cps[0:32, :],
                        wft_t[:, c * 32:(c + 1) * 32],
                        xv(c, fg * G, G),
                        start=(c == 0), stop=(c == 1))
                nc.vector.tensor_copy(f_t[fg][0:32, :], cps[0:32, :])

            # --- main attention loop (software-pipelined) ---
            stages = []
            for g in range(NIG):
                jt = 0
                for si, nt in enumerate(SUPERS):
                    stages.append((g, si, jt, nt))
                    jt += nt
            NS = len(stages)

            po_t = {}
            sps_t = {}
            eb_t = {}
            rd_t = {}

            def emit_mm1(idx):
                g, si, jt, nt = stages[idx]
                sps = SPOOLS[idx % 2].tile([128, nt * G], f32,
                                           name=SNAMES[idx % 2])
                sps_t[idx] = sps
                for t in range(nt):
                    nc.tensor.matmul(
                        sps[:, t * G:(t + 1) * G],
                        g_aug[0:33, (jt + t) * 128:(jt + t + 1) * 128],
                        f_t[g][0:33, :],
                        start=True, stop=True)

            def emit_exp(idx):
                g, si, jt, nt = stages[idx]
                eb = epool.tile([128, 3 * G], f32r, name="eb")
                eb_t[idx] = eb
                nc.scalar.activation(
                    eb[:, 0:nt * G], sps_t[idx][:, 0:nt * G], Exp)

            def emit_mm2(idx):
                g, si, jt, nt = stages[idx]
                eb = eb_t.pop(idx)
                sps_t.pop(idx)
                if si == 0:
                    po_t[g] = pso.tile([128, G], f32, name="o")
                for t in range(nt):
                    nc.tensor.matmul(
                        po_t[g][0:33, :],
                        hpt[:, (jt + t) * 33:(jt + t) * 33 + 33],
                        eb[:, t * G:(t + 1) * G],
                        start=(jt + t == 0), stop=(jt + t == NJT - 1))

            def emit_tail_recip(g):
                rd = spool.tile([128, G], f32r, name="rd")
                if g < 3:
                    nc.vector.memset(rd[:].bitcast(f32), 0.0)
                with nc.allow_low_precision(reason="softmax denom"):
                    nc.vector.reciprocal(rd[0:1, :], po_t[g][32:33, :])
                rd_t[g] = rd

            def emit_tail_pe(g, k):
                po = po_t.pop(g)
                rd = rd_t.pop(g)
                bc = pst.tile([128, G], f32, name="tl")
                nc.tensor.matmul(bc[0:32, :], e0_t, rd[:, :],
                                 start=True, stop=True)
                bcs = spool.tile([128, G], f32r, name="bcs")
                nc.vector.tensor_copy(bcs[0:32, :], bc[0:32, :])

                att = spool.tile([128, G], f32r, name="att")
                nc.vector.tensor_tensor(att[0:32, :], po[0:32, :],
                                        bcs[0:32, :], mult)
                if g < 3:
                    nc.vector.memset(att[32:64, :].bitcast(f32), 1.0)

                # output conv (gamma*Wo + bias row) -> + x residual.
                # pf reuses the "o" bank (just freed by att) so the tail
                # never steals a super-pool slot from the mm1 pipeline.
                rt = rpool.tile([128, 1024], f32, name="rt")
                out_v = res.rearrange("(c p) (gg n) -> p gg c n",
                                      c=2, n=G)[:, g]
                for c in range(2):
                    pf = (pst.tile([128, G], f32, name="tl") if c == 0
                          else pso.tile([128, G], f32, name="o"))
                    nc.tensor.matmul(
                        pf[:, :],
                        wot_t[:, c * 128:(c + 1) * 128],
                        att[0:64, :], start=True, stop=True)
                    nc.vector.tensor_tensor(
                        rt[:, c * G:(c + 1) * G], pf[:, :],
                        xv(c, g * G, G).bitcast(f32), add)
                    nc.sync.dma_start(out_v[:, c], rt[:, c * G:(c + 1) * G])

            # Pipeline: mm1[k+1] issues before mm2[k]; gh-conv groups
            # trickle in between igrp-0 stages (DMA-gated anyway); tail PE
            # work is delayed one stage so the reciprocal chain never
            # stalls the PE queue head.
            convs_left = list(range(1, 8))
            f_left = list(range(1, NIG))
            pending_tail = []
            emit_gh_conv(0)
            emit_f_conv(0, pool=pso, name="o")
            emit_mm1(0)
            import os
            KN_FSI = int(os.environ.get("KN_FSI", "6"))
            KN_MM1 = os.environ.get("KN_MM1", "mid")
            KN_TDL = int(os.environ.get("KN_TDL", "3"))
            KN_CAH = int(os.environ.get("KN_CAH", "8"))
            for k in range(NS):
                emit_exp(k)
                g, si, jt, nt = stages[k]
                if KN_MM1 == "early" and k + 1 < NS:
                    emit_mm1(k + 1)
                if g == 0:
                    need = min((jt + nt + KN_CAH) // 4, 7)
                    while convs_left and convs_left[0] <= need:
                        emit_gh_conv(convs_left.pop(0))
                if f_left and si >= KN_FSI and f_left[0] <= g + 1:
                    emit_f_conv(f_left.pop(0))
                if KN_MM1 == "mid" and k + 1 < NS:
                    emit_mm1(k + 1)
                if pending_tail and k >= pending_tail[0][1] + KN_TDL:
                    gg, kk = pending_tail.pop(0)
                    emit_tail_pe(gg, k)
                emit_mm2(k)
                if KN_MM1 == "late" and k + 1 < NS:
                    emit_mm1(k + 1)
                if si == len(SUPERS) - 1:
                    emit_tail_recip(g)
                    pending_tail.append((g, k))
            while convs_left:
                emit_gh_conv(convs_left.pop(0))
            while f_left:
                emit_f_conv(f_left.pop(0))
            while pending_tail:
                gg, kk = pending_tail.pop(0)
                emit_tail_pe(gg, kk + 2)

    nc.compile()
    return nc


def _host_prep(Wv, bv, Wk, bk, Wq, bq, Wo, bo, gamma):
    gam = float(np.asarray(gamma).reshape(-1)[0])

    # stacked gh conv weights: rows = [Wk(32); bv@Wk(1); pad(31); Wq(32)]
    w_gh = np.zeros((96, 256), np.float32)
    w_gh[0:32] = Wk
    w_gh[32] = bv @ Wk
    w_gh[64:96] = Wq
    wght = np.zeros((128, 192), np.float32)
    for c in range(2):
        wght[:, c * 96:(c + 1) * 96] = w_gh.T[c * 128:(c + 1) * 128, :]

    wft = np.zeros((128, 64), np.float32)
    for c in range(2):
        wft[:, c * 32:(c + 1) * 32] = Wv.T[c * 128:(c + 1) * 128, :]

    # output conv lhsT rows k: k<32 -> gamma*Wo^T, k==32 -> bias row
    bof = gam * (Wo @ bq + bo)                                  # [256]
    wot = np.zeros((64, 256), np.float32)
    for c in range(2):
        wot[0:32, c * 128:(c + 1) * 128] = gam * Wo[c * 128:(c + 1) * 128, :].T
        wot[32, c * 128:(c + 1) * 128] = bof[c * 128:(c + 1) * 128]

    wpk = np.zeros((128, 1600), np.float32)
    wpk[:, 0:192] = wght
    wpk[:, 1216:1280] = wft
    wpk[0:64, 1280:1536] = wot
    wpk[0, 1536:1568] = 1.0                    # e0: ones row
    wpk[0:32, 1568:1600] = np.eye(32)          # idm
    return wpk


def kernel(**inputs):
    from concourse.bass_utils import run_bass_kernel_spmd

    x = np.asarray(inputs["x"], np.float32)
    consts = _host_prep(
        np.asarray(inputs["Wv"], np.float32),
        np.asarray(inputs["bv"], np.float32),
        np.asarray(inputs["Wk"], np.float32),
        np.asarray(inputs["bk"], np.float32),
        np.asarray(inputs["Wq"], np.float32),
        np.asarray(inputs["bq"], np.float32),
        np.asarray(inputs["Wo"], np.float32),
        np.asarray(inputs["bo"], np.float32),
        np.asarray(inputs["gamma"], np.float32),
    )

    if "nc" not in _CACHE:
        _CACHE["nc"] = build_program()
    nc = _CACHE["nc"]

    in_maps = []
    for core in range(NCORES):
        b, ih = core // 2, core % 2
        # rotate keys so this core's query half sits at columns 0..NH-1
        # (softmax is invariant to key order, so this is exact), then pack
        # [weights | x] in the kernel's piece-major SBUF layout
        xrot = np.roll(x[b], -ih * NH, axis=1)
        xp = (xrot.reshape(2, 128, 8, 512)
              .transpose(1, 2, 0, 3).reshape(128, 8, 1024))
        xw = np.empty((128, 1600 + 7 * 1024), np.float32)
        xw[:, 0:1600] = consts
        xw[:, 192:1216] = xp[:, 0]
        xw[:, 1600:] = xp[:, 1:].reshape(128, 7 * 1024)
        in_maps.append({"xw": xw})

    r = run_bass_kernel_spmd(nc, in_maps, core_ids=list(range(NCORES)),
                             trace=False)
    out = np.empty((B, C, N), np.float32)
    for core in range(NCORES):
        b, ih = core // 2, core % 2
        out[b][:, ih * NH:(ih + 1) * NH] = r.results[core]["res"]
    return out


if __name__ == "__main__":
    nc = build_program()
    print("program built ok")



# revision 36
# speedup vs baseline: 1.0772x; 1.0772x over previous
"""Trainium2 Bass kernel for nn_AttentionLayer (B=4, C=256, N=4096, CR=32).

Sharding: 8 cores = (batch b in 0..3) x (query-half ih in 0..1).
Each core receives x[b] rotated so its own query half sits at columns
0..2047 (softmax is invariant to key order, so the rotation is exact);
it computes out[b][:, ih*2048:(ih+1)*2048] and the host reassembles.

Per-core algorithm:
  - g conv (keys + gbv bias-correction row) in f32r: [33,C] @ x -> g
  - h^T conv: lhsT = x chunk (stationary), rhs = Wq^T bf16 (moving,
    ap=32) -> h^T j-tiles directly in [j, 32] layout (no PE transpose)
  - f conv (queries, own half) in f32r; f/g stored as fp8e4m3 in SBUF
  - scores via fp8 DoubleRow matmul with a broadcast (stride-0) slot
    dim on both operands: psum = 2*(g_aug^T f_aug), 0.5 cycles/row.
    The 2x is undone inside exp (scale=0.5).
  - exp split across three engines: ACT native Exp; DVE/Pool compute
    Schraudolph bits = round(s*64/ln2 + B) written as int16 == bf16.
  - mm2 swapped: lhsT = eb (stationary bf16), rhs = hpt [j,33] bf16
    (moving, ap=33) accumulating num^T/den in [i, 33] psum chunks.
  - tail per i-chunk: rden = 1/den (per-partition), att^T = po*rden
    (bf16; row 32 becomes den*rden ~= 1 and doubles as the out-conv
    bias-ones row), PE transpose (bf16 identity), out conv
    (gamma*Wo^T | bias row), residual add fused into PSUM->SBUF copy.
"""

import os
import numpy as np

B, C, N = 4, 256, 4096
CR = 32
NH = N // 2          # queries per core
G = 512              # i-group width
NCORES = 8

NJT = N // 128       # 32 j-tiles
NIG = NH // G        # 4 i-groups
SUP = 2              # j-tiles per pipeline stage
NST = NJT // SUP     # stages per i-group (16)

# xw (f32r) layout: true-f32r data, DMA TF32 rounding is acceptable
W_WG = 0             # g conv lhsT   [128, 66]  (2 chunks x 33)
W_P0 = 66            # x piece 0     [128, 1024]
W_WF = 1090          # f conv lhsT   [128, 64]  (2 chunks x 32)
W_WQ = 1154          # wqt f32r      [128, 2x32]
W_IDR = 1218         # idr f32r identity [128, 128]
W_CON = 1346         # end of consts
WTOT = W_CON + 7 * 1024
# xc (f32, bit-exact DMA) layout: bit-packed bf16 constants
C_WO = 0             # wotb bf16 [33, 256] packed as u32 [33, 128]
C_ID = 128           # idm128 bf16 [128, 128] packed as u32 [128, 64]
C_TOT = 192

_CACHE = {}


def build_program():
    import concourse.bacc as bacc
    import concourse.mybir as mybir
    from concourse.tile import TileContext

    dt = mybir.dt
    f32 = dt.float32
    f32r = dt.float32r
    bf16 = dt.bfloat16
    fp8 = dt.float8e4
    i16 = dt.int16
    Exp = mybir.ActivationFunctionType.Exp
    add = mybir.AluOpType.add
    mult = mybir.AluOpType.mult
    DR = mybir.MatmulPerfMode.DoubleRow

    A_SCH = 64.0 / np.log(2.0)          # schraudolph slope on 2s input
    B_SCH = 127.0 * 128.0 - 7.0 + 0.5   # bias incl +0.5 for truncation

    nc = bacc.Bacc("TRN2", target_bir_lowering=False, debug=False,
                   num_devices=NCORES)

    # xw is f32r (DMA rounds to TF32 - fine for x and real weights); the
    # bit-packed bf16 constants ride in xc as plain f32 (bit-exact DMA).
    xw = nc.dram_tensor("xw", [128, WTOT], f32r, kind="ExternalInput").ap()
    xc = nc.dram_tensor("xc", [128, C_TOT], f32, kind="ExternalInput").ap()
    res = nc.dram_tensor("res", [C, NH], f32, kind="ExternalOutput").ap()
    DBG = os.environ.get("KN_DEBUG", "") == "1"
    if DBG:
        dbg_g = nc.dram_tensor("dbg_g", [33, N], f32, kind="ExternalOutput").ap()
        dbg_f = nc.dram_tensor("dbg_f", [33, G], f32, kind="ExternalOutput").ap()
        dbg_h = nc.dram_tensor("dbg_h", [128, NJT * 33], f32, kind="ExternalOutput").ap()
        dbg_eb = nc.dram_tensor("dbg_eb", [128, SUP * G], f32, kind="ExternalOutput").ap()
        dbg_po = nc.dram_tensor("dbg_po", [128, 136], f32, kind="ExternalOutput").ap()
        dbg_asc = nc.dram_tensor("dbg_asc", [128, 136], f32, kind="ExternalOutput").ap()
        dbg_att = nc.dram_tensor("dbg_att", [33, 512], f32, kind="ExternalOutput").ap()
        dbg_wot = nc.dram_tensor("dbg_wot", [33, 256], f32, kind="ExternalOutput").ap()
        dbg_op = nc.dram_tensor("dbg_op", [128, 1024], f32, kind="ExternalOutput").ap()

    # exp engine schedule per stage: A=ACT native exp, D=DVE schraudolph
    sched = os.environ.get("KN_EXP", "AAD" * 22)
    assert len(sched) >= NIG * NST

    with TileContext(nc) as tc:
        with (
            tc.tile_pool(name="const", bufs=1) as cpool,
            tc.tile_pool(name="eb", bufs=6) as epool,
            tc.tile_pool(name="small", bufs=2) as spool,
            tc.tile_pool(name="resp", bufs=2) as rpool,
            tc.tile_pool(name="psS", bufs=2, space="PSUM") as psS,
            tc.tile_pool(name="psC", bufs=1, space="PSUM") as psC,
            tc.tile_pool(name="psP", bufs=1, space="PSUM") as psP,
            tc.tile_pool(name="psO", bufs=1, space="PSUM") as psO,
        ):
            # --- weights + x in one tile; DMA0 carries consts+piece0 ---
            xall = cpool.tile([128, WTOT], f32r)
            xcs = cpool.tile([128, C_TOT], f32)
            wg_t = xall[:, W_WG:W_WG + 66]
            wft_t = xall[:, W_WF:W_WF + 64]
            wqt_t = xall[:, W_WQ:W_WQ + 64]                     # [128, 64]
            idr_t = xall[:, W_IDR:W_IDR + 128]                  # [128,128]
            wot_t = xcs[0:33, C_WO:C_WO + 128].bitcast(bf16)    # [33, 256]
            idm_t = xcs[:, C_ID:C_ID + 64].bitcast(bf16)        # [128, 128]
            nc.sync.dma_start(xall[:, 0:W_WF], xw[:, 0:W_WF])
            nc.sync.dma_start(xall[:, W_WF:W_CON], xw[:, W_WF:W_CON])
            nc.sync.dma_start(xcs[:, :], xc[:, :])
            for gp in range(1, 8):
                s0 = W_CON + (gp - 1) * 1024
                nc.sync.dma_start(xall[:, s0:s0 + 1024], xw[:, s0:s0 + 1024])

            def xv(c, col, w):
                # x chunk c (c in 0..1), columns [col, col+w) piece-major
                gp = col // G
                assert col % G + w <= G
                base = W_P0 if gp == 0 else W_CON + (gp - 1) * 1024
                return xall[:, base + c * G + col % G:
                            base + c * G + col % G + w]

            # --- activation buffers ---
            f_t = []
            for gi in range(NIG):
                ft = cpool.tile([33, G], fp8, name=f"f{gi}")
                f_t.append(ft)
                nc.vector.memset(ft[32:33, :], 1.0)
            g_aug = cpool.tile([33, N], fp8)      # rows: g(32), gbv(1)
            hpt = cpool.tile([128, NJT * 33], bf16)
            hpt_v = hpt[:].rearrange("p (t w) -> p t w", w=33)
            # num^T/den accumulators: both ig parities in one PSUM bank
            po_all = psP.tile([128, 2 * 4 * 34], f32, name="po")

            # --- g conv: [33, 512] per group ---
            def emit_g_conv(grp):
                cps = psC.tile([33, G], f32, name="cv")
                for c in range(2):
                    nc.tensor.matmul(
                        cps[:, :],
                        wg_t[:, c * 33:(c + 1) * 33],
                        xv(c, grp * G, G),
                        start=(c == 0), stop=(c == 1))
                nc.vector.tensor_copy(
                    g_aug[:, grp * G:(grp + 1) * G], cps[:, :])

            # --- h^T conv: 4 j-tiles per group, out [j, 32] directly ---
            def emit_h_conv(grp):
                hps = psC.tile([128, 4 * 32], f32, name="cv", tag="cv")
                for k in range(4):
                    jt = 4 * grp + k
                    for c in range(2):
                        nc.tensor.matmul(
                            hps[:, k * 32:(k + 1) * 32],
                            xv(c, jt * 128, 128),
                            wqt_t[:, c * 32:(c + 1) * 32],
                            start=(c == 0), stop=(c == 1))
                nc.vector.tensor_copy(
                    hpt_v[:, 4 * grp:4 * grp + 4, 0:32],
                    hps[:].rearrange("p (t w) -> p t w", w=32))
                nc.vector.memset(hpt_v[:, 4 * grp:4 * grp + 4, 32:33], 1.0)

            # --- f conv (own query half): f = Wv @ xq, fp8 out ---
            def emit_f_conv(fg):
                cps = psC.tile([32, G], f32, name="cv", tag="cv")
                for c in range(2):
                    nc.tensor.matmul(
                        cps[:, :],
                        wft_t[:, c * 32:(c + 1) * 32],
                        xv(c, fg * G, G),
                        start=(c == 0), stop=(c == 1))
                nc.vector.tensor_copy(f_t[fg][0:32, :], cps[:, :])

            # --- main attention loop ---
            stages = [(g, si) for g in range(NIG) for si in range(NST)]
            NS = len(stages)

            po_t = {}
            att_t = {}
            sps_t = {}
            eb_t = {}
            rd_t = {}

            def emit_mm1(idx):
                g, si = stages[idx]
                sps = psS.tile([128, SUP * G], f32, name="s")
                sps_t[idx] = sps
                fr = f_t[g][:, :].unsqueeze(1).broadcast_to([33, 2, G])
                for t in range(SUP):
                    jt = si * SUP + t
                    gl = (g_aug[:, jt * 128:(jt + 1) * 128]
                          .unsqueeze(1).broadcast_to([33, 2, 128]))
                    nc.tensor.matmul(
                        sps[:, t * G:(t + 1) * G], gl, fr,
                        start=True, stop=True, perf_mode=DR)

            def emit_exp(idx):
                eng = sched[idx]
                eb = epool.tile([128, SUP * G], bf16, name="eb")
                eb_t[idx] = eb
                sps = sps_t.pop(idx)
                if eng == "A":
                    nc.scalar.activation(eb[:, :], sps[:, :], Exp, scale=0.5)
                else:
                    e = nc.vector if eng == "D" else nc.gpsimd
                    e.tensor_scalar(eb[:, :].bitcast(i16), sps[:, :],
                                    A_SCH, B_SCH, mult, add)
                if DBG and idx == 0:
                    t = rpool.tile([128, SUP * G], f32, name="dbe")
                    nc.vector.tensor_copy(t[:, :], eb[:, :])
                    nc.sync.dma_start(dbg_eb, t[:, :])

            def emit_mm2(idx):
                g, si = stages[idx]
                eb = eb_t.pop(idx)
                if si == 0:
                    po_t[g] = po_all[:, (g % 2) * 136:(g % 2) * 136 + 136]
                for t in range(SUP):
                    jt = si * SUP + t
                    for c in range(4):
                        nc.tensor.matmul(
                            po_t[g][:, c * 34:c * 34 + 33],
                            eb[:, (t * 4 + c) * 128:(t * 4 + c + 1) * 128],
                            hpt_v[:, jt],
                            start=(jt == 0), stop=(jt == NJT - 1))

            # --- tail: scale -> transpose -> out conv (+x via PE) ---
            def emit_tail_scale(g):
                # rden for all 4 chunks in one strided reciprocal, then
                # att^T[i, 0:33] bf16 = po * rden (row 32 -> ~1.0, which
                # doubles as the out-conv bias-ones row)
                rd = spool.tile([128, 4], f32, name="rd")
                pv = po_t[g][:, :].rearrange("p (c w) -> p c w", w=34)
                with nc.allow_low_precision(reason="softmax denom"):
                    nc.vector.reciprocal(rd[:, :], pv[:, :, 32])
                asc = spool.tile([128, 4 * 34], bf16, name="asc")
                nc.vector.tensor_tensor(
                    asc[:, :].rearrange("p (c w) -> p c w", w=34),
                    pv[:, :, :],
                    rd[:, :].unsqueeze(2).broadcast_to([128, 4, 34]),
                    mult)
                rd_t[g] = (rd, asc)
                if DBG and g == 0:
                    t = rpool.tile([128, 136], f32, name="dbp")
                    nc.vector.tensor_copy(t[:, :], po_t[g][:, :])
                    nc.sync.dma_start(dbg_po, t[:, :])
                    t2 = rpool.tile([128, 136], f32, name="dba")
                    nc.vector.tensor_copy(t2[:, :], asc[:, :])
                    nc.sync.dma_start(dbg_asc, t2[:, :])

            def emit_tail_tps(g):
                _, asc = rd_t[g]
                atp = psC.tile([33, 512], bf16, name="cv", tag="cv")
                for c in range(4):
                    nc.tensor.transpose(
                        atp[:, c * 128:(c + 1) * 128],
                        asc[:, c * 34:c * 34 + 33], idm_t)
                att = spool.tile([33, 512], bf16, name="att")
                nc.vector.tensor_copy(att[:, :], atp[:, :])
                att_t[g] = att
                if DBG and g == 0:
                    t = rpool.tile([33, 512], f32, name="dbt")
                    nc.vector.tensor_copy(t[:, :], att[:, :])
                    nc.sync.dma_start(dbg_att, t[:, :])

            KN_RESID = os.environ.get("KN_RESID", "pe")

            def emit_tail_conv(g):
                att = att_t[g]
                op = psO.tile([128, 1024], f32, name="o")
                att_t[g] = op
                for cc in range(2):
                    if KN_RESID == "pe":
                        nc.tensor.matmul(
                            op[:, cc * G:(cc + 1) * G], idr_t,
                            xv(cc, g * G, G),
                            start=True, stop=False, skip_group_check=True)
                    nc.tensor.matmul(
                        op[:, cc * G:(cc + 1) * G],
                        wot_t[:, cc * 128:(cc + 1) * 128],
                        att[:, :], start=(KN_RESID != "pe"), stop=True,
                        skip_group_check=True)

            def emit_tail_out(g):
                po_t.pop(g)
                rd_t.pop(g)
                op = att_t.pop(g)
                rt = rpool.tile([128, 1024], f32, name="rt")
                out_v = res.rearrange("(c p) (gg n) -> p gg c n",
                                      c=2, n=G)[:, g]
                if DBG and g == 0:
                    t = rpool.tile([128, 1024], f32, name="dbo")
                    nc.vector.tensor_copy(t[:, :], op[:, :])
                    nc.sync.dma_start(dbg_op, t[:, :])
                    t2 = rpool.tile([33, 256], f32, name="dbw")
                    nc.vector.tensor_copy(t2[:, :], wot_t[:, :])
                    nc.sync.dma_start(dbg_wot, t2[:, :])
                if KN_RESID == "pe":
                    nc.vector.tensor_copy(rt[:, :], op[:, :])
                else:
                    for cc in range(2):
                        nc.vector.tensor_tensor(
                            rt[:, cc * G:(cc + 1) * G],
                            op[:, cc * G:(cc + 1) * G],
                            xv(cc, g * G, G).bitcast(f32), add)
                for cc in range(2):
                    nc.sync.dma_start(out_v[:, cc], rt[:, cc * G:(cc + 1) * G])

            # --- pipeline ---
            KN_CAH = int(os.environ.get("KN_CAH", "6"))
            KN_FSI = int(os.environ.get("KN_FSI", "11"))
            KN_TDL = int(os.environ.get("KN_TDL", "1"))
            convs_left = list(range(1, 8))
            f_left = list(range(1, NIG))
            pending = []   # (g, tail_step, stage_done)

            emit_g_conv(0)
            emit_h_conv(0)
            emit_f_conv(0)
            emit_mm1(0)
            for k in range(NS):
                g, si = stages[k]
                emit_exp(k)
                if g == 0:
                    need = min(((si + 1) * SUP + KN_CAH) // 4, 7)
                    while convs_left and convs_left[0] <= need:
                        gp = convs_left.pop(0)
                        emit_g_conv(gp)
                        emit_h_conv(gp)
                if f_left and si >= KN_FSI and f_left[0] <= g + 1:
                    emit_f_conv(f_left.pop(0))
                if k + 1 < NS:
                    emit_mm1(k + 1)
                TAIL = [emit_tail_scale, emit_tail_tps,
                        emit_tail_conv, emit_tail_out]
                if pending and k >= pending[0][2] + KN_TDL:
                    gg, step, _ = pending[0]
                    TAIL[step](gg)
                    if step == 3:
                        pending.pop(0)
                    else:
                        pending[0] = (gg, step + 1, pending[0][2])
                emit_mm2(k)
                if si == NST - 1:
                    pending.append((g, 0, k))
            while convs_left:
                gp = convs_left.pop(0)
                emit_g_conv(gp)
                emit_h_conv(gp)
            while f_left:
                emit_f_conv(f_left.pop(0))
            while pending:
                gg, step, _ = pending.pop(0)
                for st in range(step, 4):
                    [emit_tail_scale, emit_tail_tps,
                     emit_tail_conv, emit_tail_out][st](gg)
            if DBG:
                t = rpool.tile([33, N], f32, name="dbg")
                nc.vector.tensor_copy(t[:, :], g_aug[:, :])
                nc.sync.dma_start(dbg_g, t[:, :])
                t = rpool.tile([33, G], f32, name="dbf")
                nc.vector.tensor_copy(t[:, :], f_t[0][:, :])
                nc.sync.dma_start(dbg_f, t[:, :])
                t = rpool.tile([128, NJT * 33], f32, name="dbh")
                nc.vector.tensor_copy(t[:, :], hpt[:, :])
                nc.sync.dma_start(dbg_h, t[:, :])

    nc.compile()
    return nc


def _host_prep(Wv, bv, Wk, bk, Wq, bq, Wo, bo, gamma):
    import ml_dtypes
    bfd = ml_dtypes.bfloat16
    gam = float(np.asarray(gamma).reshape(-1)[0])

    # g conv lhsT: rows [Wk(32); bv@Wk(1)]
    w_g = np.zeros((33, 256), np.float32)
    w_g[0:32] = Wk
    w_g[32] = bv @ Wk
    wg = np.zeros((128, 66), np.float32)
    for c in range(2):
        wg[:, c * 33:(c + 1) * 33] = w_g.T[c * 128:(c + 1) * 128, :]

    wft = np.zeros((128, 64), np.float32)
    for c in range(2):
        wft[:, c * 32:(c + 1) * 32] = Wv.T[c * 128:(c + 1) * 128, :]

    # out conv lhsT rows k: k<32 -> gamma*Wo^T, k==32 -> bias row (bf16)
    bof = gam * (Wo @ bq + bo)                                  # [256]
    wot = np.zeros((33, 256), np.float32)
    for c in range(2):
        wot[0:32, c * 128:(c + 1) * 128] = gam * Wo[c * 128:(c + 1) * 128, :].T
        wot[32, c * 128:(c + 1) * 128] = bof[c * 128:(c + 1) * 128]

    wqt = np.zeros((128, 64), np.float32)   # bf16 [128, 2x32]
    for c in range(2):
        wqt[:, c * 32:(c + 1) * 32] = Wq.T[c * 128:(c + 1) * 128, :]

    def pack_bf16(a):
        u = a.astype(bfd).view(np.uint16).astype(np.uint32)
        return (u[:, 0::2] | (u[:, 1::2] << 16)).view(np.float32)

    wpk = np.zeros((128, W_CON), np.float32)
    wpk[:, W_WG:W_WG + 66] = wg
    wpk[:, W_WF:W_WF + 64] = wft
    wpk[:, W_WQ:W_WQ + 64] = wqt
    wpk[:, W_IDR:W_IDR + 128] = np.eye(128, dtype=np.float32)
    cpk = np.zeros((128, C_TOT), np.float32)
    cpk[0:33, C_WO:C_WO + 128] = pack_bf16(wot)
    cpk[:, C_ID:C_ID + 64] = pack_bf16(np.eye(128, dtype=np.float32))
    return wpk, cpk


def kernel(**inputs):
    from concourse.bass_utils import run_bass_kernel_spmd

    x = np.asarray(inputs["x"], np.float32)
    consts, cpk = _host_prep(
        np.asarray(inputs["Wv"], np.float32),
        np.asarray(inputs["bv"], np.float32),
        np.asarray(inputs["Wk"], np.float32),
        np.asarray(inputs["bk"], np.float32),
        np.asarray(inputs["Wq"], np.float32),
        np.asarray(inputs["bq"], np.float32),
        np.asarray(inputs["Wo"], np.float32),
        np.asarray(inputs["bo"], np.float32),
        np.asarray(inputs["gamma"], np.float32),
    )

    if "nc" not in _CACHE:
        _CACHE["nc"] = build_program()
    nc = _CACHE["nc"]

    in_maps = []
    for core in range(NCORES):
        b, ih = core // 2, core % 2
        xrot = np.roll(x[b], -ih * NH, axis=1)
        xp = (xrot.reshape(2, 128, 8, 512)
              .transpose(1, 2, 0, 3).reshape(128, 8, 1024))
        xwb = np.empty((128, WTOT), np.float32)
        xwb[:, 0:W_CON] = consts
        xwb[:, W_P0:W_P0 + 1024] = xp[:, 0]
        xwb[:, W_CON:] = xp[:, 1:].reshape(128, 7 * 1024)
        in_maps.append({"xw": xwb, "xc": cpk})

    r = run_bass_kernel_spmd(nc, in_maps, core_ids=list(range(NCORES)),
                             trace=False)
    out = np.empty((B, C, N), np.float32)
    for core in range(NCORES):
        b, ih = core // 2, core % 2
        out[b][:, ih * NH:(ih + 1) * NH] = r.results[core]["res"]
    return out


if __name__ == "__main__":
    nc = build_program()
    print("program built ok")


# revision 70
# speedup vs baseline: 1.3250x; 1.2300x over previous
"""Trainium2 Bass kernel for nn_AttentionLayer (B=4, C=256, N=4096, CR=32).

Sharding: 8 cores = (batch b in 0..3) x (query-half ih in 0..1).
Each core receives x[b] rotated so its own query half sits at columns
0..2047 (softmax is invariant to key order, so the rotation is exact);
it computes out[b][:, ih*2048:(ih+1)*2048] and the host reassembles.

Per-core algorithm:
  - g conv (keys + gbv bias-correction row) in f32r: [33,C] @ x -> g
  - h^T conv: lhsT = x chunk (stationary), rhs = Wq^T bf16 (moving,
    ap=32) -> h^T j-tiles directly in [j, 32] layout (no PE transpose)
  - f conv (queries, own half) in f32r; f/g stored as fp8e4m3 in SBUF
  - scores via fp8 DoubleRow matmul with a broadcast (stride-0) slot
    dim on both operands: psum = 2*(g_aug^T f_aug), 0.5 cycles/row.
    The 2x is undone inside exp (scale=0.5).
  - exp split across three engines: ACT native Exp; DVE/Pool compute
    Schraudolph bits = round(s*64/ln2 + B) written as int16 == bf16.
  - mm2 swapped: lhsT = eb (stationary bf16), rhs = hpt [j,33] bf16
    (moving, ap=33) accumulating num^T/den in [i, 33] psum chunks.
  - tail per i-chunk: rden = 1/den (per-partition), att^T = po*rden
    (bf16; row 32 becomes den*rden ~= 1 and doubles as the out-conv
    bias-ones row), PE transpose (bf16 identity), out conv
    (gamma*Wo^T | bias row), residual add fused into PSUM->SBUF copy.
"""

import os
import numpy as np

B, C, N = 4, 256, 4096
CR = 32
NH = N // 2          # queries per core
G = 512              # i-group width
NCORES = 8

NJT = N // 128       # 32 j-tiles
NIG = NH // G        # 4 i-groups
SUP = 2              # j-tiles per pipeline stage
NST = NJT // SUP     # stages per i-group (16)

# xw (f32r) layout: true-f32r data, DMA TF32 rounding is acceptable
W_WG = 0             # g conv lhsT   [128, 66]  (2 chunks x 33)
W_P0 = 66            # x piece 0     [128, 1024]
W_WF = 1090          # f conv lhsT   [128, 64]  (2 chunks x 32)
W_WQ = 1154          # wqt f32r      [128, 2x32]
W_IDR = 1218         # idr f32r identity [128, 128]
W_CON = 1346         # end of consts
WTOT = W_CON + 7 * 1024
# xc (f32, bit-exact DMA) layout: bit-packed bf16 constants
C_WO = 0             # wotb bf16 [33, 256] packed as u32 [33, 128]
C_ID = 128           # idm128 bf16 [128, 128] packed as u32 [128, 64]
C_TOT = 192

_CACHE = {}


def build_program():
    import concourse.bacc as bacc
    import concourse.mybir as mybir
    from concourse.tile import TileContext

    dt = mybir.dt
    f32 = dt.float32
    f32r = dt.float32r
    bf16 = dt.bfloat16
    fp8 = dt.float8e4
    i16 = dt.int16
    Exp = mybir.ActivationFunctionType.Exp
    add = mybir.AluOpType.add
    mult = mybir.AluOpType.mult
    DR = mybir.MatmulPerfMode.DoubleRow

    A_SCH = 64.0 / np.log(2.0)          # schraudolph slope on 2s input
    B_SCH = 127.0 * 128.0 - 7.0 + 0.5   # bias incl +0.5 for truncation

    nc = bacc.Bacc("TRN2", target_bir_lowering=False, debug=False,
                   num_devices=NCORES)

    # xw is f32r (DMA rounds to TF32 - fine for x and real weights); the
    # bit-packed bf16 constants ride in xc as plain f32 (bit-exact DMA).
    xw = nc.dram_tensor("xw", [128, WTOT], f32r, kind="ExternalInput").ap()
    xc = nc.dram_tensor("xc", [128, C_TOT], f32, kind="ExternalInput").ap()
    res = nc.dram_tensor("res", [C, NH], f32, kind="ExternalOutput").ap()
    DBG = os.environ.get("KN_DEBUG", "") == "1"
    if DBG:
        dbg_g = nc.dram_tensor("dbg_g", [33, N], f32, kind="ExternalOutput").ap()
        dbg_f = nc.dram_tensor("dbg_f", [33, G], f32, kind="ExternalOutput").ap()
        dbg_h = nc.dram_tensor("dbg_h", [128, NJT * 33], f32, kind="ExternalOutput").ap()
        dbg_eb = nc.dram_tensor("dbg_eb", [128, SUP * G], f32, kind="ExternalOutput").ap()
        dbg_po = nc.dram_tensor("dbg_po", [128, 136], f32, kind="ExternalOutput").ap()
        dbg_asc = nc.dram_tensor("dbg_asc", [128, 136], f32, kind="ExternalOutput").ap()
        dbg_att = nc.dram_tensor("dbg_att", [33, 512], f32, kind="ExternalOutput").ap()
        dbg_wot = nc.dram_tensor("dbg_wot", [33, 256], f32, kind="ExternalOutput").ap()
        dbg_op = nc.dram_tensor("dbg_op", [128, 1024], f32, kind="ExternalOutput").ap()

    # exp engine schedule per stage: A=ACT native exp, D=DVE schraudolph.
    # ig0 keeps DVE mostly free for conv copies; later igs alternate more.
    sched = os.environ.get(
        "KN_EXP", "A" * 16 + "AADADAADADAADADA" * 3)
    assert len(sched) >= NIG * NST

    with TileContext(nc) as tc:
        with (
            tc.tile_pool(name="const", bufs=1) as cpool,
            tc.tile_pool(name="eb", bufs=6) as epool,
            tc.tile_pool(name="small", bufs=2) as spool,
            tc.tile_pool(name="resp", bufs=2) as rpool,
            tc.tile_pool(name="psS", bufs=3, space="PSUM") as psS,
            tc.tile_pool(name="psC", bufs=1, space="PSUM") as psC,
            tc.tile_pool(name="psP", bufs=1, space="PSUM") as psP,
        ):
            # --- weights + x in one tile; DMA0 carries consts+piece0 ---
            xall = cpool.tile([128, WTOT], f32r)
            xcs = cpool.tile([128, C_TOT], f32)
            wg_t = xall[:, W_WG:W_WG + 66]
            wft_t = xall[:, W_WF:W_WF + 64]
            wqt_t = xall[:, W_WQ:W_WQ + 64]                     # [128, 64]
            idr_t = xall[:, W_IDR:W_IDR + 128]                  # [128,128]
            wot_t = xcs[0:33, C_WO:C_WO + 128].bitcast(bf16)    # [33, 256]
            idm_t = xcs[:, C_ID:C_ID + 64].bitcast(bf16)        # [128, 128]
            # all consts + piece0 in one SP HWDGE transfer (transfers are
            # serialized; anything split off would land behind piece 1);
            # remaining pieces via gpsimd SWDGE
            nc.sync.dma_start(xall[:, 0:W_CON], xw[:, 0:W_CON])
            nc.scalar.dma_start(xcs[:, :], xc[:, :])
            for gp in range(1, 8):
                s0 = W_CON + (gp - 1) * 1024
                nc.gpsimd.dma_start(xall[:, s0:s0 + 1024], xw[:, s0:s0 + 1024])

            def xv(c, col, w):
                # x chunk c (c in 0..1), columns [col, col+w) piece-major
                gp = col // G
                assert col % G + w <= G
                base = W_P0 if gp == 0 else W_CON + (gp - 1) * 1024
                return xall[:, base + c * G + col % G:
                            base + c * G + col % G + w]

            # --- activation buffers ---
            f_t = []
            for gi in range(NIG):
                ft = cpool.tile([33, G], fp8, name=f"f{gi}")
                f_t.append(ft)
                nc.vector.memset(ft[32:33, :], 1.0)
            g_aug = cpool.tile([33, N], fp8)      # rows: g(32), gbv(1)
            hpt = cpool.tile([128, NJT * 33], bf16)
            hpt_v = hpt[:].rearrange("p (t w) -> p t w", w=33)
            # one hand-packed PSUM bank: num^T/den accumulators for both
            # ig parities (cols 0:272) + h^T conv scratch (cols 288:416)
            pack = psP.tile([128, 512], f32, name="po")
            po_all = pack[:, 0:272]

            # --- g conv: [33, 256] half-groups on tag cva ---
            def emit_g_half(grp, h):
                col0 = h * 256
                cps = psC.tile([33, 256], f32, name="cva", tag="cva")
                for c in range(2):
                    nc.tensor.matmul(
                        cps[:, :],
                        wg_t[:, c * 33:(c + 1) * 33],
                        xv(c, grp * G + col0, 256),
                        start=(c == 0), stop=(c == 1))
                nc.vector.tensor_copy(
                    g_aug[:, grp * G + col0:grp * G + col0 + 256], cps[:, :])

            def emit_g_conv(grp):
                emit_g_half(grp, 0)
                emit_g_half(grp, 1)

            # --- h^T conv: 4 j-tiles per group on tag cvh (parallel) ---
            def emit_h_conv(grp):
                hps = pack[:, 288:416]
                for k in range(4):
                    jt = 4 * grp + k
                    for c in range(2):
                        nc.tensor.matmul(
                            hps[:, k * 32:(k + 1) * 32],
                            xv(c, jt * 128, 128),
                            wqt_t[:, c * 32:(c + 1) * 32],
                            start=(c == 0), stop=(c == 1))
                nc.vector.tensor_copy(
                    hpt_v[:, 4 * grp:4 * grp + 4, 0:32],
                    hps.rearrange("p (t w) -> p t w", w=32))
                nc.vector.memset(hpt_v[:, 4 * grp:4 * grp + 4, 32:33], 1.0)

            # --- f conv (own query half) in halves, fp8 out ---
            def emit_f_half(fg, h):
                col0 = h * 256
                cps = psC.tile([32, 256], f32, name="cva", tag="cva")
                for c in range(2):
                    nc.tensor.matmul(
                        cps[:, :],
                        wft_t[:, c * 32:(c + 1) * 32],
                        xv(c, fg * G + col0, 256),
                        start=(c == 0), stop=(c == 1))
                nc.vector.tensor_copy(
                    f_t[fg][0:32, col0:col0 + 256], cps[:, :])

            def emit_f_conv(fg):
                emit_f_half(fg, 0)
                emit_f_half(fg, 1)

            # --- main attention loop ---
            stages = [(g, si) for g in range(NIG) for si in range(NST)]
            NS = len(stages)

            po_t = {}
            att_t = {}
            sps_t = {}
            eb_t = {}
            rd_t = {}
            op_t = {}
            rt_t = {}

            def emit_mm1(idx):
                g, si = stages[idx]
                sps = psS.tile([128, SUP * G], f32, name="s")
                sps_t[idx] = sps
                fr = f_t[g][:, :].unsqueeze(1).broadcast_to([33, 2, G])
                for t in range(SUP):
                    jt = si * SUP + t
                    gl = (g_aug[:, jt * 128:(jt + 1) * 128]
                          .unsqueeze(1).broadcast_to([33, 2, 128]))
                    nc.tensor.matmul(
                        sps[:, t * G:(t + 1) * G], gl, fr,
                        start=True, stop=True, perf_mode=DR)

            def emit_exp(idx):
                eng = sched[idx]
                eb = epool.tile([128, SUP * G], bf16, name="eb")
                eb_t[idx] = eb
                sps = sps_t.pop(idx)
                if eng == "A":
                    nc.scalar.activation(eb[:, :], sps[:, :], Exp, scale=0.5)
                else:
                    e = nc.vector if eng == "D" else nc.gpsimd
                    e.tensor_scalar(eb[:, :].bitcast(i16), sps[:, :],
                                    A_SCH, B_SCH, mult, add)
                if DBG and idx == 0:
                    t = rpool.tile([128, SUP * G], f32, name="dbe")
                    nc.vector.tensor_copy(t[:, :], eb[:, :])
                    nc.sync.dma_start(dbg_eb, t[:, :])

            def emit_mm2(idx):
                g, si = stages[idx]
                eb = eb_t.pop(idx)
                if si == 0:
                    po_t[g] = po_all[:, (g % 2) * 136:(g % 2) * 136 + 136]
                for t in range(SUP):
                    jt = si * SUP + t
                    for c in range(4):
                        nc.tensor.matmul(
                            po_t[g][:, c * 34:c * 34 + 33],
                            eb[:, (t * 4 + c) * 128:(t * 4 + c + 1) * 128],
                            hpt_v[:, jt],
                            start=(jt == 0), stop=(jt == NJT - 1))

            # --- tail: scale -> transpose -> out conv (+x via PE) ---
            def emit_tail_scale(g):
                # rden for all 4 chunks in one strided reciprocal, then
                # att^T[i, 0:33] bf16 = po * rden (row 32 -> ~1.0, which
                # doubles as the out-conv bias-ones row)
                rd = spool.tile([128, 4], f32, name="rd")
                pv = po_t[g][:, :].rearrange("p (c w) -> p c w", w=34)
                with nc.allow_low_precision(reason="softmax denom"):
                    nc.vector.reciprocal(rd[:, :], pv[:, :, 32])
                asc = spool.tile([128, 4 * 34], bf16, name="asc")
                nc.vector.tensor_tensor(
                    asc[:, :].rearrange("p (c w) -> p c w", w=34),
                    pv[:, :, :],
                    rd[:, :].unsqueeze(2).broadcast_to([128, 4, 34]),
                    mult)
                rd_t[g] = (rd, asc)
                if DBG and g == 0:
                    t = rpool.tile([128, 136], f32, name="dbp")
                    nc.vector.tensor_copy(t[:, :], po_t[g][:, :])
                    nc.sync.dma_start(dbg_po, t[:, :])
                    t2 = rpool.tile([128, 136], f32, name="dba")
                    nc.vector.tensor_copy(t2[:, :], asc[:, :])
                    nc.sync.dma_start(dbg_asc, t2[:, :])

            def emit_tail_tps(g):
                _, asc = rd_t[g]
                atp = psC.tile([33, 512], bf16, name="cva", tag="cva")
                for c in range(4):
                    nc.tensor.transpose(
                        atp[:, c * 128:(c + 1) * 128],
                        asc[:, c * 34:c * 34 + 33], idm_t)
                att = spool.tile([33, 512], bf16, name="att")
                nc.vector.tensor_copy(att[:, :], atp[:, :])
                att_t[g] = att
                if DBG and g == 0:
                    t = rpool.tile([33, 512], f32, name="dbt")
                    nc.vector.tensor_copy(t[:, :], att[:, :])
                    nc.sync.dma_start(dbg_att, t[:, :])

            KN_RESID = os.environ.get("KN_RESID", "pe")

            def emit_tail_conv(g, cc):
                att = att_t[g]
                if cc == 0:
                    op = psS.tile([128, 1024], f32, name="s", tag="s")
                    op_t[g] = op
                op = op_t[g]
                nc.tensor.matmul(
                    op[:, cc * G:(cc + 1) * G], idr_t,
                    xv(cc, g * G, G),
                    start=True, stop=False, skip_group_check=True)
                nc.tensor.matmul(
                    op[:, cc * G:(cc + 1) * G],
                    wot_t[:, cc * 128:(cc + 1) * 128],
                    att[:, :], start=False, stop=True,
                    skip_group_check=True)

            def emit_tail_out(g, cc):
                op = op_t[g]
                if cc == 0:
                    rt_t[g] = rpool.tile([128, 1024], f32, name="rt")
                rt = rt_t[g]
                out_v = res.rearrange("(c p) (gg n) -> p gg c n",
                                      c=2, n=G)[:, g]
                if DBG and g == 0 and cc == 0:
                    t = rpool.tile([128, 1024], f32, name="dbo")
                    nc.vector.tensor_copy(t[:, :], op[:, :])
                    nc.sync.dma_start(dbg_op, t[:, :])
                    t2 = rpool.tile([33, 256], f32, name="dbw")
                    nc.vector.tensor_copy(t2[:, :], wot_t[:, :])
                    nc.sync.dma_start(dbg_wot, t2[:, :])
                if g == NIG - 1 and cc == 1:
                    # last tail: ACT is idle; overlap the two rt copies
                    nc.scalar.copy(rt[:, cc * G:(cc + 1) * G],
                                   op[:, cc * G:(cc + 1) * G])
                else:
                    nc.vector.tensor_copy(rt[:, cc * G:(cc + 1) * G],
                                          op[:, cc * G:(cc + 1) * G])
                nc.sync.dma_start(out_v[:, cc], rt[:, cc * G:(cc + 1) * G])
                if cc == 1:
                    po_t.pop(g)
                    rd_t.pop(g)
                    att_t.pop(g)
                    op_t.pop(g)
                    rt_t.pop(g)

            # --- pipeline ---
            KN_CAH = int(os.environ.get("KN_CAH", "6"))
            KN_FSI = int(os.environ.get("KN_FSI", "11"))
            KN_TDL = int(os.environ.get("KN_TDL", "1"))
            convs_left = list(range(1, 8))
            f_left = list(range(1, NIG))
            pending = []   # (g, tail_step, stage_done)

            TAIL = [lambda g: emit_tail_scale(g),
                    lambda g: emit_tail_tps(g),
                    lambda g: emit_tail_conv(g, 0),
                    lambda g: emit_tail_conv(g, 1),
                    lambda g: emit_tail_out(g, 0),
                    lambda g: emit_tail_out(g, 1)]
            NTS = len(TAIL)

            emit_f_conv(0)
            emit_g_half(0, 0)
            emit_mm1(0)
            emit_g_half(0, 1)
            emit_mm1(1)
            emit_h_conv(0)
            for k in range(NS):
                g, si = stages[k]
                emit_exp(k)
                if k + 2 < NS:
                    emit_mm1(k + 2)
                if g == 0:
                    need = min(((si + 1) * SUP + KN_CAH) // 4, 7)
                    while convs_left and convs_left[0] <= need:
                        gp = convs_left.pop(0)
                        emit_g_conv(gp)
                        emit_h_conv(gp)
                if f_left and si >= KN_FSI and f_left[0] <= g + 1:
                    emit_f_conv(f_left.pop(0))
                if pending and k >= pending[0][2] + KN_TDL:
                    gg, step, _ = pending[0]
                    TAIL[step](gg)
                    if step == NTS - 1:
                        pending.pop(0)
                    else:
                        pending[0] = (gg, step + 1, pending[0][2])
                emit_mm2(k)
                if si == NST - 1:
                    pending.append((g, 0, k))
            while convs_left:
                gp = convs_left.pop(0)
                emit_g_conv(gp)
                emit_h_conv(gp)
            while f_left:
                emit_f_conv(f_left.pop(0))
            while pending:
                gg, step, _ = pending.pop(0)
                for st in range(step, NTS):
                    TAIL[st](gg)
            if DBG:
                t = rpool.tile([33, N], f32, name="dbg")
                nc.vector.tensor_copy(t[:, :], g_aug[:, :])
                nc.sync.dma_start(dbg_g, t[:, :])
                t = rpool.tile([33, G], f32, name="dbf")
                nc.vector.tensor_copy(t[:, :], f_t[0][:, :])
                nc.sync.dma_start(dbg_f, t[:, :])
                t = rpool.tile([128, NJT * 33], f32, name="dbh")
                nc.vector.tensor_copy(t[:, :], hpt[:, :])
                nc.sync.dma_start(dbg_h, t[:, :])

    nc.compile()
    return nc


def _host_prep(Wv, bv, Wk, bk, Wq, bq, Wo, bo, gamma):
    import ml_dtypes
    bfd = ml_dtypes.bfloat16
    gam = float(np.asarray(gamma).reshape(-1)[0])

    # g conv lhsT: rows [Wk(32); bv@Wk(1)]
    w_g = np.zeros((33, 256), np.float32)
    w_g[0:32] = Wk
    w_g[32] = bv @ Wk
    wg = np.zeros((128, 66), np.float32)
    for c in range(2):
        wg[:, c * 33:(c + 1) * 33] = w_g.T[c * 128:(c + 1) * 128, :]

    wft = np.zeros((128, 64), np.float32)
    for c in range(2):
        wft[:, c * 32:(c + 1) * 32] = Wv.T[c * 128:(c + 1) * 128, :]

    # out conv lhsT rows k: k<32 -> gamma*Wo^T, k==32 -> bias row (bf16)
    bof = gam * (Wo @ bq + bo)                                  # [256]
    wot = np.zeros((33, 256), np.float32)
    for c in range(2):
        wot[0:32, c * 128:(c + 1) * 128] = gam * Wo[c * 128:(c + 1) * 128, :].T
        wot[32, c * 128:(c + 1) * 128] = bof[c * 128:(c + 1) * 128]

    wqt = np.zeros((128, 64), np.float32)   # bf16 [128, 2x32]
    for c in range(2):
        wqt[:, c * 32:(c + 1) * 32] = Wq.T[c * 128:(c + 1) * 128, :]

    def pack_bf16(a):
        u = a.astype(bfd).view(np.uint16).astype(np.uint32)
        return (u[:, 0::2] | (u[:, 1::2] << 16)).view(np.float32)

    wpk = np.zeros((128, W_CON), np.float32)
    wpk[:, W_WG:W_WG + 66] = wg
    wpk[:, W_WF:W_WF + 64] = wft
    wpk[:, W_WQ:W_WQ + 64] = wqt
    wpk[:, W_IDR:W_IDR + 128] = np.eye(128, dtype=np.float32)
    cpk = np.zeros((128, C_TOT), np.float32)
    cpk[0:33, C_WO:C_WO + 128] = pack_bf16(wot)
    cpk[:, C_ID:C_ID + 64] = pack_bf16(np.eye(128, dtype=np.float32))
    return wpk, cpk


def kernel(**inputs):
    from concourse.bass_utils import run_bass_kernel_spmd

    x = np.asarray(inputs["x"], np.float32)
    consts, cpk = _host_prep(
        np.asarray(inputs["Wv"], np.float32),
        np.asarray(inputs["bv"], np.float32),
        np.asarray(inputs["Wk"], np.float32),
        np.asarray(inputs["bk"], np.float32),
        np.asarray(inputs["Wq"], np.float32),
        np.asarray(inputs["bq"], np.float32),
        np.asarray(inputs["Wo"], np.float32),
        np.asarray(inputs["bo"], np.float32),
        np.asarray(inputs["gamma"], np.float32),
    )

    if "nc" not in _CACHE:
        _CACHE["nc"] = build_program()
    nc = _CACHE["nc"]

    in_maps = []
    for core in range(NCORES):
        b, ih = core // 2, core % 2
        xrot = np.roll(x[b], -ih * NH, axis=1)
        xp = (xrot.reshape(2, 128, 8, 512)
              .transpose(1, 2, 0, 3).reshape(128, 8, 1024))
        xwb = np.empty((128, WTOT), np.float32)
        xwb[:, 0:W_CON] = consts
        xwb[:, W_P0:W_P0 + 1024] = xp[:, 0]
        xwb[:, W_CON:] = xp[:, 1:].reshape(128, 7 * 1024)
        in_maps.append({"xw": xwb, "xc": cpk})

    r = run_bass_kernel_spmd(nc, in_maps, core_ids=list(range(NCORES)),
                             trace=False)
    out = np.empty((B, C, N), np.float32)
    for core in range(NCORES):
        b, ih = core // 2, core % 2
        out[b][:, ih * NH:(ih + 1) * NH] = r.results[core]["res"]
    return out


if __name__ == "__main__":
    nc = build_program()
    print("program built ok")
